# revision 31
# baseline (speedup 1.0000x reference)
"""Trainium2 Bass kernel for CustomAttention (qkv -> per-head LN on q,k -> SDPA -> proj).

Sharding: 8 cores = 2 batches x 4 head-groups (3 heads each).

v4 structure:
- Phase B for heads 0+1 together (384-row qkv matmuls); raw/v copies split
  across DVE and the otherwise-idle ACT engine.
- LN per head finishes k first (Pool mu/rstd passes split by type), so the
  32 k-transposes can run while q's LN completes; q-transposes are
  interleaved into the attention i-block loop (only the next i-block's
  columns are needed).
- Head 2's qkv matmuls interleave into head 0's attention; its LN runs
  during attention. Output projection (bf16) interleaves into head 2's
  attention loop.
- ACT runs exp on [128,1024] PSUM tiles; PSUM: scores 2x2 banks + PV 2x1 +
  transpose/filler pools.
"""

import os
import sys
from functools import lru_cache

import numpy as np

for _p in ("/opt/trn_rl_repo", os.path.expanduser("~/.axon_site/_ro/trn_rl_repo")):
    if os.path.isdir(_p) and _p not in sys.path:
        sys.path.insert(0, _p)

import concourse.bass as bass
import concourse.mybir as mybir
from concourse import bacc
import concourse.tile as tile
from concourse.masks import make_identity

F32 = mybir.dt.float32
F32R = mybir.dt.float32r
BF16 = mybir.dt.bfloat16
ALU = mybir.AluOpType
ACTF = mybir.ActivationFunctionType

H = 3          # heads per core
D = 64         # head dim
C = 768        # model dim
J = 3 * H * D  # qkv rows per core = 576
EPS = 1e-5
SCALE = D ** -0.5

SKEW = 6       # PV matmuls lag scores by this many 512-wide chunks
GRP = 2        # score j-chunks per PSUM/exp tile


def build_nc(N=4096):
    """One-core program; all 8 cores run it SPMD with different input data."""
    NB = N // 128          # n-blocks / j-chunks = 32
    IB = N // 512          # i-blocks = 8
    NHALF = NB // 2

    nc = bacc.Bacc("TRN2", target_bir_lowering=False, debug=False)
    x_t = nc.declare_dram_parameter("x_t", [C, N], BF16, isOutput=False)
    # host layout: [C, (h, q|k|v, 64)] = per-head column groups
    wqkv_t = nc.declare_dram_parameter("wqkv_t", [C, J], BF16, isOutput=False)
    projw_t = nc.declare_dram_parameter("projw_t", [H * D, C], BF16, isOutput=False)
    gb = nc.declare_dram_parameter("gb", [4, D], F32, isOutput=False)
    out_p = nc.declare_dram_parameter("out_p", [N, C], F32, isOutput=True)

    with tile.TileContext(nc) as tc:
        with (
            tc.tile_pool(name="persist", bufs=1) as persist,
            tc.tile_pool(name="weights", bufs=1) as weights,
            tc.tile_pool(name="raw", bufs=2) as rawp,
            tc.tile_pool(name="stats", bufs=2) as statsp,
        ):
            # ---- persistent SBUF tensors ----
            # qT duplicated across both partition halves: rows 0:64 == 64:128
            qT = [persist.tile([128, N], BF16, tag=f"qT{h}", name=f"qT{h}") for h in range(H)]
            # kT stacked: rows 0:64 = j in [0,N/2), rows 64:128 = j in [N/2,N)
            kT = [persist.tile([128, N // 2], BF16, tag=f"kT{h}", name=f"kT{h}") for h in range(H)]
            # V augmented with a ones column (index 64) per j-chunk
            vA = [persist.tile([128, NB, 65], BF16, tag=f"vA{h}", name=f"vA{h}") for h in range(H)]
            # attention output, channel-major: ao1 rows = h0,h1; ao2 rows = h2
            ao1 = persist.tile([128, N], BF16, tag="ao1")
            ao2 = persist.tile([64, N], BF16, tag="ao2")

            for h in range(H):
                nc.vector.memset(vA[h][:, :, 64:65], 1.0)

            wq = weights.tile([128, 6, J], BF16, tag="wqkv")
            nc.sync.dma_start(
                wq[:], wqkv_t.rearrange("(ck p) j -> p ck j", p=128)
            )
            pw128 = weights.tile([128, C], BF16, tag="pw128")
            nc.sync.dma_start(pw128[:], projw_t[0:128, :])
            pw64 = weights.tile([64, C], BF16, tag="pw64")
            nc.sync.dma_start(pw64[:], projw_t[128:192, :])
            # gamma/beta broadcast across partitions: rows [gq*s, bq*s, gk, bk]
            gbt = weights.tile([128, 4, D], F32, tag="gb")
            nc.sync.dma_start(gbt[:], gb[None, :, :].to_broadcast([128, 4, D]))
            gam2 = weights.tile([128, 2, D], BF16, tag="gam2")
            nc.vector.tensor_copy(gam2[:, 0, :], gbt[:, 0, :])
            nc.vector.tensor_copy(gam2[:, 1, :], gbt[:, 2, :])
            bet2 = weights.tile([128, 2, D], BF16, tag="bet2")
            nc.vector.tensor_copy(bet2[:, 0, :], gbt[:, 1, :])
            nc.vector.tensor_copy(bet2[:, 1, :], gbt[:, 3, :])
            epst = weights.tile([128, 1], F32, tag="epst")
            nc.vector.memset(epst[:], EPS)
            ident = weights.tile([128, 128], F32, tag="ident")
            make_identity(nc, ident[:])
            identb = weights.tile([128, 128], BF16, tag="identb")
            nc.vector.tensor_copy(identb[:], ident[:])

            rawqs = {}
            rawks = {}
            qdups = {}

            def ln_stats(h):
                """Batched LN stats for head h; mu/rstd -> bf16.

                q slab is [128, NB, D]; k slab is [128, NHALF, 2, D] in
                (column-block, j-half) order so each kT 128-column block is
                one 2D DMA transpose. Stats index 1 follows k's layout."""
                rawq, rawk = rawqs[h], rawks[h]
                s1 = statsp.tile([128, 2, NB], F32, tag="s1")
                s2 = statsp.tile([128, 2, NB], F32, tag="s2")
                sq = rawp.tile([128, 2, NB, D], BF16, tag="sq", name=f"sq{h}")
                kv = rawk[:].rearrange("p c j d -> p (c j) d")
                nc.vector.tensor_reduce(
                    s1[:, 0, :], rawq[:], mybir.AxisListType.X, ALU.add
                )
                nc.vector.tensor_reduce(s1[:, 1, :], kv, mybir.AxisListType.X, ALU.add)
                nc.vector.tensor_mul(sq[:, 0], rawq[:], rawq[:])
                nc.vector.tensor_mul(sq[:, 1], kv, kv)
                nc.vector.tensor_reduce(s2[:], sq[:], mybir.AxisListType.X, ALU.add)
                mu = statsp.tile([128, 2, NB], F32, tag="mu")
                nc.vector.tensor_scalar_mul(mu[:], s1[:], 1.0 / D)
                var = statsp.tile([128, 2, NB], F32, tag="var")
                nc.vector.tensor_scalar_mul(var[:], s2[:], 1.0 / D)
                musq = statsp.tile([128, 2, NB], F32, tag="musq")
                nc.vector.tensor_mul(musq[:], mu[:], mu[:])
                nc.vector.tensor_sub(var[:], var[:], musq[:])
                std = statsp.tile([128, 2, NB], F32, tag="std")
                nc.scalar.activation(std[:], var[:], ACTF.Sqrt, bias=epst[:])
                rstd = statsp.tile([128, 2, NB], F32, tag="rstd")
                nc.vector.reciprocal(rstd[:], std[:])
                # one Newton step: r <- r*(1.5 - 0.5*(var+eps)*r^2)
                nr = statsp.tile([128, 2, NB], F32, tag="nr")
                nc.vector.tensor_mul(nr[:], rstd[:], rstd[:])
                ve = statsp.tile([128, 2, NB], F32, tag="ve")
                nc.vector.tensor_scalar_add(ve[:], var[:], EPS)
                nc.vector.tensor_mul(nr[:], nr[:], ve[:])
                nc.vector.tensor_scalar(nr[:], nr[:], -0.5, 1.5, ALU.mult, ALU.add)
                nc.vector.tensor_mul(rstd[:], rstd[:], nr[:])
                mu_b = statsp.tile([128, 2, NB], BF16, tag="mu_b", name=f"mu_b{h}")
                nc.vector.tensor_copy(mu_b[:], mu[:])
                rstd_b = statsp.tile([128, 2, NB], BF16, tag="rstd_b", name=f"rstd_b{h}")
                nc.vector.tensor_copy(rstd_b[:], rstd[:])
                return mu_b, rstd_b

            def ln_finish_k(h, mu_b, rstd_b):
                kk = rawks[h][:].rearrange("p c j d -> p (c j) d")
                mu4 = mu_b[:, 1, :, None].broadcast_to([128, NB, D])
                rs4 = rstd_b[:, 1, :, None].broadcast_to([128, NB, D])
                nc.gpsimd.tensor_sub(kk, kk, mu4)
                nc.gpsimd.tensor_mul(kk, kk, rs4)
                gk = gam2[:, 1, None, :].broadcast_to([128, NB, D])
                bk = bet2[:, 1, None, :].broadcast_to([128, NB, D])
                nc.vector.tensor_mul(kk, kk, gk)
                nc.vector.tensor_add(kk, kk, bk)

            def ln_finish_q(h, mu_b, rstd_b):
                qq = rawqs[h][:]
                mu4 = mu_b[:, 0, :, None].broadcast_to([128, NB, D])
                rs4 = rstd_b[:, 0, :, None].broadcast_to([128, NB, D])
                nc.gpsimd.tensor_sub(qq, qq, mu4)
                nc.gpsimd.tensor_mul(qq, qq, rs4)
                qdup = rawp.tile([128, NB, 2, D], BF16, tag="qdup", name=f"qdup{h}")
                gq = gam2[:, 0, None, :].broadcast_to([128, NB, D])
                bq = bet2[:, 0, None, :].broadcast_to([128, NB, D])
                nc.vector.tensor_mul(qdup[:, :, 0, :], qq, gq)
                nc.vector.tensor_add(qdup[:, :, 0, :], qdup[:, :, 0, :], bq)
                nc.vector.tensor_copy(qdup[:, :, 1, :], qdup[:, :, 0, :])
                qdups[h] = qdup

            def alloc_raw(h):
                rawqs[h] = rawp.tile([128, NB, D], BF16, tag="rawq", name=f"rawq{h}")
                rawks[h] = rawp.tile(
                    [128, NHALF, 2, D], BF16, tag="rawk", name=f"rawk{h}"
                )

            def b_copies(h, ps, nb, qk_off, eng):
                """psum -> raw slabs; k goes to its (cb, jh) slot."""
                jh, cb = nb // NHALF, nb % NHALF
                eng(rawqs[h][:, nb, :], ps[:, qk_off : qk_off + 64])
                eng(rawks[h][:, cb, jh, :], ps[:, qk_off + 64 : qk_off + 128])
                eng(vA[h][:, nb, 0:64], ps[:, qk_off + 128 : qk_off + 192])

            def phase_b01():
                """qkv for heads 0 and 1 (one pass over x)."""
                alloc_raw(0)
                alloc_raw(1)
                with (
                    tc.tile_pool(name="pB01", bufs=4) as pB,
                    tc.tile_pool(name="psB01", bufs=4, space="PSUM") as psB,
                ):
                    for nb in range(NB):
                        xt = pB.tile([128, 6, 128], BF16, tag="xt")
                        nc.sync.dma_start(
                            xt[:],
                            x_t.rearrange("(ck p) n -> p ck n", p=128)[
                                :, :, nb * 128 : (nb + 1) * 128
                            ],
                        )
                        ps = psB.tile([128, 384], F32, tag="qkvps")
                        for ck in range(6):
                            nc.tensor.matmul(
                                ps[:],
                                xt[:, ck, :],
                                wq[:, ck, 0:384],
                                start=(ck == 0),
                                stop=(ck == 5),
                            )
                        b_copies(0, ps, nb, 0, nc.vector.tensor_copy)
                        # head 1's copies on the idle ACT engine
                        b_copies(1, ps, nb, 192, nc.scalar.copy)

            def phase_b2_mm(pB, psB, nb_lo, nb_hi):
                """Head 2 qkv matmuls + copies for a range of nb (filler)."""
                for nb in range(nb_lo, nb_hi):
                    xt = pB.tile([128, 6, 128], BF16, tag="xt2")
                    nc.sync.dma_start(
                        xt[:],
                        x_t.rearrange("(ck p) n -> p ck n", p=128)[
                            :, :, nb * 128 : (nb + 1) * 128
                        ],
                    )
                    ps = psB.tile([128, 192], F32, tag="qkvps2")
                    for ck in range(6):
                        nc.tensor.matmul(
                            ps[:],
                            xt[:, ck, :],
                            wq[:, ck, 384:576],
                            start=(ck == 0),
                            stop=(ck == 5),
                        )
                    b_copies(2, ps, nb, 0, nc.vector.tensor_copy)

            def t_k(h):
                """All kT column blocks via 2D DMA transposes (one per cb:
                rawk's (cb, jh, d) block transposes straight into kT's
                stacked layout)."""
                for cb in range(NHALF):
                    nc.sync.dma_start_transpose(
                        kT[h][:, cb * 128 : (cb + 1) * 128],
                        rawks[h][:, cb, :, :],
                    )

            def t_q(h, nb_lo, nb_hi):
                """qT blocks via 2D DMA transposes from the duplicated slab."""
                qdup = qdups[h]
                for nb in range(nb_lo, min(nb_hi, NB)):
                    nc.sync.dma_start_transpose(
                        qT[h][:, nb * 128 : (nb + 1) * 128],
                        qdup[:, nb, :, :],
                    )

            def proj_nb(psD, pD, nb_lo, nb_hi):
                """Output projection for a range of n-blocks (bf16)."""
                for nb in range(nb_lo, nb_hi):
                    blk = slice(nb * 128, (nb + 1) * 128)
                    stage = pD.tile([128, C], F32, tag="stage")
                    for oc, osz in ((0, 512), (512, 256)):
                        ps = psD.tile([128, 512], F32, tag="pd")
                        nc.tensor.matmul(
                            ps[:, 0:osz],
                            ao1[:, blk],
                            pw128[:, oc : oc + osz],
                            start=True,
                            stop=False,
                        )
                        nc.tensor.matmul(
                            ps[:, 0:osz],
                            ao2[0:64, blk],
                            pw64[0:64, oc : oc + osz],
                            start=False,
                            stop=True,
                        )
                        nc.vector.tensor_copy(stage[:, oc : oc + osz], ps[:, 0:osz])
                    nc.sync.dma_start(out_p[blk, :], stage[:])

            def phase_c(h, ptp, pCs, psS, psO, ib_hook=None):
                """Full attention for head h; ib_hook(ib) emits filler PE work.

                The score/PV PSUM pools are shared across heads so no pool
                boundary (which waits on ALL prior readers, including trailing
                exps) sits between consecutive heads."""
                if True:
                    ngrp = (NB + GRP - 1) // GRP
                    for ib in range(IB):
                        isl = slice(ib * 512, (ib + 1) * 512)
                        pso = psO.tile([65, 512], F32, tag="pso")
                        queue = []
                        n_pv = [0]

                        def emit_pv(pso=pso, queue=queue, n_pv=n_pv, h=h):
                            pt_half, jc = queue.pop(0)
                            nc.tensor.matmul(
                                pso[:],
                                vA[h][:, jc, :],
                                pt_half,
                                start=(n_pv[0] == 0),
                                stop=(n_pv[0] == NB - 1),
                            )
                            n_pv[0] += 1

                        for g in range(ngrp):
                            # pair chunks (g, g+NHALF): alternating PE tile
                            # rows lets the next LDWEIGHTS overlap the current
                            # matmul in the other quadrant
                            chunks = [g, g + NHALF]
                            w = len(chunks)
                            ps = psS.tile([128, GRP, 512], F32, tag="st")
                            for s in range(w):
                                jc = chunks[s]
                                jh, cb = jc // NHALF, jc % NHALF
                                psl = slice(64 * jh, 64 * jh + 64)
                                nc.tensor.matmul(
                                    ps[:, s, :],
                                    kT[h][psl, cb * 128 : (cb + 1) * 128],
                                    qT[h][psl, isl],
                                    start=True,
                                    stop=True,
                                    tile_position=(64 * jh, 0),
                                )
                            pt = ptp.tile([128, GRP, 512], BF16, tag="pt")
                            nc.scalar.activation(
                                pt[:, 0:w, :], ps[:, 0:w, :], ACTF.Exp
                            )
                            for s in range(w):
                                queue.append((pt[:, s, :], chunks[s]))
                            while len(queue) > SKEW:
                                emit_pv()
                        while queue:
                            emit_pv()

                        rden_f = pCs.tile([1, 512], F32, tag="rden_f")
                        nc.vector.tensor_copy(rden_f[:], pso[64:65, :])
                        rden = pCs.tile([1, 512], F32, tag="rden")
                        nc.vector.reciprocal_approx_fast(rden[:], rden_f[:])
                        rb = pCs.tile([64, 512], F32, tag="rb")
                        nc.gpsimd.partition_broadcast(rb[:], rden[:])
                        if h == 0:
                            nc.vector.tensor_mul(ao1[0:64, isl], pso[0:64, :], rb[:])
                        elif h == 2:
                            nc.vector.tensor_mul(ao2[0:64, isl], pso[0:64, :], rb[:])
                        else:
                            stg = pCs.tile([64, 512], BF16, tag="stg")
                            nc.vector.tensor_mul(stg[:], pso[0:64, :], rb[:])
                            nc.sync.dma_start(ao1[64:128, isl], stg[:])
                        if ib_hook is not None:
                            ib_hook(ib)

            # ================= emission =================
            phase_b01()
            mrs0 = ln_stats(0)
            ln_finish_k(0, *mrs0)
            ln_finish_q(0, *mrs0)
            alloc_raw(2)
            t_q(0, 0, 4)
            t_k(0)
            t_q(0, 4, NB)

            # shared attention pools: psS 4 banks + psO 2; psB2 1 during C0,
            # psD 2 during C2
            with (
                tc.tile_pool(name="ptC", bufs=4) as ptp,
                tc.tile_pool(name="pCs", bufs=4) as pCs,
                tc.tile_pool(name="psS", bufs=2, space="PSUM") as psS,
                tc.tile_pool(name="psO", bufs=2, space="PSUM") as psO,
            ):
                with (
                    tc.tile_pool(name="pB2", bufs=4) as pB2,
                    tc.tile_pool(name="psB2", bufs=1, space="PSUM") as psB2,
                ):
                    def hook0(ib):
                        if ib == 0:
                            mrs1 = ln_stats(1)
                            ln_finish_k(1, *mrs1)
                            ln_finish_q(1, *mrs1)
                        elif ib == 1:
                            t_q(1, 0, 4)
                            t_k(1)
                            t_q(1, 4, NB)
                        phase_b2_mm(pB2, psB2, ib * 4, ib * 4 + 4)
                    phase_c(0, ptp, pCs, psS, psO, ib_hook=hook0)

                # head 2 LN + transposes overlap C1
                mrs2 = ln_stats(2)
                ln_finish_k(2, *mrs2)
                ln_finish_q(2, *mrs2)

                def hook1(ib):
                    if ib == 1:
                        t_q(2, 0, 4)
                        t_k(2)
                        t_q(2, 4, NB)
                phase_c(1, ptp, pCs, psS, psO, ib_hook=hook1)

                with (
                    tc.tile_pool(name="pD", bufs=3) as pD,
                    tc.tile_pool(name="psD", bufs=2, space="PSUM") as psD,
                ):
                    def hook2(ib):
                        proj_nb(psD, pD, ib * 4, ib * 4 + 4)
                    phase_c(2, ptp, pCs, psS, psO, ib_hook=hook2)

    nc.compile()
    return nc


@lru_cache(maxsize=2)
def _built(N):
    nc = build_nc(N)
    return nc


def _prep_inputs(x, qkv_w, q_gamma, q_beta, k_gamma, k_beta, proj_w):
    x = np.asarray(x, np.float32)
    qkv_w = np.asarray(qkv_w, np.float32)
    proj_w = np.asarray(proj_w, np.float32)
    B = x.shape[0]
    import ml_dtypes
    xts = [np.ascontiguousarray(x[b].T).astype(ml_dtypes.bfloat16) for b in range(B)]
    gbs = []
    wqs = []
    pws = []
    for g in range(4):
        r = slice(192 * g, 192 * (g + 1))
        qg = qkv_w[0:768][r]       # [192, 768] q rows of this group's 3 heads
        kg = qkv_w[768:1536][r]
        vg = qkv_w[1536:2304][r]
        # per-head interleave: [q_h(64) | k_h(64) | v_h(64)] x 3 heads
        blocks = []
        for h in range(3):
            hs = slice(64 * h, 64 * (h + 1))
            blocks += [qg[hs], kg[hs], vg[hs]]
        wq_rows = np.concatenate(blocks, axis=0)   # [576, 768]
        wqs.append(np.ascontiguousarray(wq_rows.T).astype(ml_dtypes.bfloat16))
        pws.append(
            np.ascontiguousarray(proj_w[:, r].T).astype(ml_dtypes.bfloat16)
        )
        gbs.append(
            np.stack(
                [
                    np.asarray(q_gamma, np.float32) * SCALE,
                    np.asarray(q_beta, np.float32) * SCALE,
                    np.asarray(k_gamma, np.float32),
                    np.asarray(k_beta, np.float32),
                ]
            )
        )
    in_maps = []
    for core in range(8):
        b, g = core // 4, core % 4
        in_maps.append(
            {"x_t": xts[b], "wqkv_t": wqs[g], "projw_t": pws[g], "gb": gbs[g]}
        )
    return in_maps


def run_cores(in_maps, N, trace=False):
    from concourse.bass_utils import run_bass_kernel_spmd

    nc = _built(N)
    res = run_bass_kernel_spmd(nc, in_maps, list(range(8)), trace=trace)
    return res


def kernel(x, qkv_w, q_gamma, q_beta, k_gamma, k_beta, proj_w, proj_b):
    x = np.asarray(x, np.float32)
    N = x.shape[1]
    in_maps = _prep_inputs(x, qkv_w, q_gamma, q_beta, k_gamma, k_beta, proj_w)
    res = run_cores(in_maps, N)
    parts = [np.asarray(r["out_p"], np.float32) for r in res.results]
    out0 = parts[0] + parts[1] + parts[2] + parts[3]
    out1 = parts[4] + parts[5] + parts[6] + parts[7]
    out = np.stack([out0, out1]) + np.asarray(proj_b, np.float32)
    return out.astype(np.float32)


# revision 33
# speedup vs baseline: 1.0480x; 1.0480x over previous
"""Trainium2 Bass kernel for CustomAttention (qkv -> per-head LN on q,k -> SDPA -> proj).

Sharding: 8 cores = 2 batches x 4 head-groups (3 heads each).

v4 structure:
- Phase B for heads 0+1 together (384-row qkv matmuls); raw/v copies split
  across DVE and the otherwise-idle ACT engine.
- LN per head finishes k first (Pool mu/rstd passes split by type), so the
  32 k-transposes can run while q's LN completes; q-transposes are
  interleaved into the attention i-block loop (only the next i-block's
  columns are needed).
- Head 2's qkv matmuls interleave into head 0's attention; its LN runs
  during attention. Output projection (bf16) interleaves into head 2's
  attention loop.
- ACT runs exp on [128,1024] PSUM tiles; PSUM: scores 2x2 banks + PV 2x1 +
  transpose/filler pools.
"""

import os
import sys
from functools import lru_cache

import numpy as np

for _p in ("/opt/trn_rl_repo", os.path.expanduser("~/.axon_site/_ro/trn_rl_repo")):
    if os.path.isdir(_p) and _p not in sys.path:
        sys.path.insert(0, _p)

import concourse.bass as bass
import concourse.mybir as mybir
from concourse import bacc
import concourse.tile as tile
from concourse.masks import make_identity

F32 = mybir.dt.float32
F32R = mybir.dt.float32r
BF16 = mybir.dt.bfloat16
ALU = mybir.AluOpType
ACTF = mybir.ActivationFunctionType

H = 3          # heads per core
D = 64         # head dim
C = 768        # model dim
J = 3 * H * D  # qkv rows per core = 576
EPS = 1e-5
SCALE = D ** -0.5

SKEW = 6       # PV matmuls lag scores by this many 512-wide chunks
GRP = 2        # score j-chunks per PSUM/exp tile


def build_nc(N=4096):
    """One-core program; all 8 cores run it SPMD with different input data."""
    NB = N // 128          # n-blocks / j-chunks = 32
    IB = N // 512          # i-blocks = 8
    NHALF = NB // 2

    nc = bacc.Bacc("TRN2", target_bir_lowering=False, debug=False)
    x_t = nc.declare_dram_parameter("x_t", [C, N], BF16, isOutput=False)
    # host layout: [C, (h, q|k|v, 64)] = per-head column groups
    wqkv_t = nc.declare_dram_parameter("wqkv_t", [C, J], BF16, isOutput=False)
    projw_t = nc.declare_dram_parameter("projw_t", [H * D, C], BF16, isOutput=False)
    gb = nc.declare_dram_parameter("gb", [4, D], F32, isOutput=False)
    out_p = nc.declare_dram_parameter("out_p", [N, C], F32, isOutput=True)

    with tile.TileContext(nc) as tc:
        with (
            tc.tile_pool(name="persist", bufs=1) as persist,
            tc.tile_pool(name="weights", bufs=1) as weights,
            tc.tile_pool(name="raw", bufs=2) as rawp,
            tc.tile_pool(name="stats", bufs=2) as statsp,
        ):
            # ---- persistent SBUF tensors ----
            # qT duplicated across both partition halves: rows 0:64 == 64:128
            qT = [persist.tile([128, N], BF16, tag=f"qT{h}", name=f"qT{h}") for h in range(H)]
            # kT stacked: rows 0:64 = j in [0,N/2), rows 64:128 = j in [N/2,N)
            kT = [persist.tile([128, N // 2], BF16, tag=f"kT{h}", name=f"kT{h}") for h in range(H)]
            # V augmented with a ones column (index 64) per j-chunk
            vA = [persist.tile([128, NB, 65], BF16, tag=f"vA{h}", name=f"vA{h}") for h in range(H)]
            # attention output, channel-major: ao1 rows = h0,h1; ao2 rows = h2
            ao1 = persist.tile([128, N], BF16, tag="ao1")
            ao2 = persist.tile([64, N], BF16, tag="ao2")

            for h in range(H):
                nc.vector.memset(vA[h][:, :, 64:65], 1.0)

            wq = weights.tile([128, 6, J], BF16, tag="wqkv")
            nc.sync.dma_start(
                wq[:], wqkv_t.rearrange("(ck p) j -> p ck j", p=128)
            )
            pw128 = weights.tile([128, C], BF16, tag="pw128")
            nc.sync.dma_start(pw128[:], projw_t[0:128, :])
            pw64 = weights.tile([64, C], BF16, tag="pw64")
            nc.sync.dma_start(pw64[:], projw_t[128:192, :])
            # gamma/beta broadcast across partitions: rows [gq*s, bq*s, gk, bk]
            gbt = weights.tile([128, 4, D], F32, tag="gb")
            nc.sync.dma_start(gbt[:], gb[None, :, :].to_broadcast([128, 4, D]))
            gam2 = weights.tile([128, 2, D], BF16, tag="gam2")
            nc.vector.tensor_copy(gam2[:, 0, :], gbt[:, 0, :])
            nc.vector.tensor_copy(gam2[:, 1, :], gbt[:, 2, :])
            bet2 = weights.tile([128, 2, D], BF16, tag="bet2")
            nc.vector.tensor_copy(bet2[:, 0, :], gbt[:, 1, :])
            nc.vector.tensor_copy(bet2[:, 1, :], gbt[:, 3, :])
            epst = weights.tile([128, 1], F32, tag="epst")
            nc.vector.memset(epst[:], EPS)
            ident = weights.tile([128, 128], F32, tag="ident")
            make_identity(nc, ident[:])
            identb = weights.tile([128, 128], BF16, tag="identb")
            nc.vector.tensor_copy(identb[:], ident[:])

            rawqs = {}
            rawks = {}
            qdups = {}

            def ln_stats(h):
                """Batched LN stats for head h; mu/rstd -> bf16.

                q slab is [128, NB, D]; k slab is [128, NHALF, 2, D] in
                (column-block, j-half) order so each kT 128-column block is
                one 2D DMA transpose. Stats index 1 follows k's layout."""
                rawq, rawk = rawqs[h], rawks[h]
                s1 = statsp.tile([128, 2, NB], F32, tag="s1")
                s2 = statsp.tile([128, 2, NB], F32, tag="s2")
                sq = rawp.tile([128, 2, NB, D], BF16, tag="sq", name=f"sq{h}")
                kv = rawk[:].rearrange("p c j d -> p (c j) d")
                nc.vector.tensor_reduce(
                    s1[:, 0, :], rawq[:], mybir.AxisListType.X, ALU.add
                )
                nc.vector.tensor_reduce(s1[:, 1, :], kv, mybir.AxisListType.X, ALU.add)
                nc.vector.tensor_mul(sq[:, 0], rawq[:], rawq[:])
                nc.vector.tensor_mul(sq[:, 1], kv, kv)
                nc.vector.tensor_reduce(s2[:], sq[:], mybir.AxisListType.X, ALU.add)
                mu = statsp.tile([128, 2, NB], F32, tag="mu")
                nc.vector.tensor_scalar_mul(mu[:], s1[:], 1.0 / D)
                var = statsp.tile([128, 2, NB], F32, tag="var")
                nc.vector.tensor_scalar_mul(var[:], s2[:], 1.0 / D)
                musq = statsp.tile([128, 2, NB], F32, tag="musq")
                nc.vector.tensor_mul(musq[:], mu[:], mu[:])
                nc.vector.tensor_sub(var[:], var[:], musq[:])
                std = statsp.tile([128, 2, NB], F32, tag="std")
                nc.scalar.activation(std[:], var[:], ACTF.Sqrt, bias=epst[:])
                rstd = statsp.tile([128, 2, NB], F32, tag="rstd")
                nc.vector.reciprocal(rstd[:], std[:])
                # one Newton step: r <- r*(1.5 - 0.5*(var+eps)*r^2)
                nr = statsp.tile([128, 2, NB], F32, tag="nr")
                nc.vector.tensor_mul(nr[:], rstd[:], rstd[:])
                ve = statsp.tile([128, 2, NB], F32, tag="ve")
                nc.vector.tensor_scalar_add(ve[:], var[:], EPS)
                nc.vector.tensor_mul(nr[:], nr[:], ve[:])
                nc.vector.tensor_scalar(nr[:], nr[:], -0.5, 1.5, ALU.mult, ALU.add)
                nc.vector.tensor_mul(rstd[:], rstd[:], nr[:])
                mu_b = statsp.tile([128, 2, NB], BF16, tag="mu_b", name=f"mu_b{h}")
                nc.vector.tensor_copy(mu_b[:], mu[:])
                rstd_b = statsp.tile([128, 2, NB], BF16, tag="rstd_b", name=f"rstd_b{h}")
                nc.vector.tensor_copy(rstd_b[:], rstd[:])
                return mu_b, rstd_b

            def ln_finish_k(h, mu_b, rstd_b):
                kk = rawks[h][:].rearrange("p c j d -> p (c j) d")
                mu4 = mu_b[:, 1, :, None].broadcast_to([128, NB, D])
                rs4 = rstd_b[:, 1, :, None].broadcast_to([128, NB, D])
                nc.gpsimd.tensor_sub(kk, kk, mu4)
                nc.gpsimd.tensor_mul(kk, kk, rs4)
                gk = gam2[:, 1, None, :].broadcast_to([128, NB, D])
                bk = bet2[:, 1, None, :].broadcast_to([128, NB, D])
                nc.vector.tensor_mul(kk, kk, gk)
                nc.vector.tensor_add(kk, kk, bk)

            def ln_finish_q(h, mu_b, rstd_b):
                qq = rawqs[h][:]
                mu4 = mu_b[:, 0, :, None].broadcast_to([128, NB, D])
                rs4 = rstd_b[:, 0, :, None].broadcast_to([128, NB, D])
                nc.gpsimd.tensor_sub(qq, qq, mu4)
                nc.gpsimd.tensor_mul(qq, qq, rs4)
                qdup = rawp.tile([128, NB, 2, D], BF16, tag="qdup", name=f"qdup{h}")
                gq = gam2[:, 0, None, :].broadcast_to([128, NB, D])
                bq = bet2[:, 0, None, :].broadcast_to([128, NB, D])
                nc.vector.tensor_mul(qdup[:, :, 0, :], qq, gq)
                nc.vector.tensor_add(qdup[:, :, 0, :], qdup[:, :, 0, :], bq)
                nc.vector.tensor_copy(qdup[:, :, 1, :], qdup[:, :, 0, :])
                qdups[h] = qdup

            def alloc_raw(h):
                rawqs[h] = rawp.tile([128, NB, D], BF16, tag="rawq", name=f"rawq{h}")
                rawks[h] = rawp.tile(
                    [128, NHALF, 2, D], BF16, tag="rawk", name=f"rawk{h}"
                )

            def b_copies(h, ps, nb, qk_off, eng):
                """psum -> raw slabs; k goes to its (cb, jh) slot."""
                jh, cb = nb // NHALF, nb % NHALF
                eng(rawqs[h][:, nb, :], ps[:, qk_off : qk_off + 64])
                eng(rawks[h][:, cb, jh, :], ps[:, qk_off + 64 : qk_off + 128])
                eng(vA[h][:, nb, 0:64], ps[:, qk_off + 128 : qk_off + 192])

            def phase_b01():
                """qkv for heads 0 and 1 (one pass over x)."""
                alloc_raw(0)
                alloc_raw(1)
                with (
                    tc.tile_pool(name="pB01", bufs=4) as pB,
                    tc.tile_pool(name="psB01", bufs=4, space="PSUM") as psB,
                ):
                    for nb in range(NB):
                        xt = pB.tile([128, 6, 128], BF16, tag="xt")
                        nc.sync.dma_start(
                            xt[:],
                            x_t.rearrange("(ck p) n -> p ck n", p=128)[
                                :, :, nb * 128 : (nb + 1) * 128
                            ],
                        )
                        ps = psB.tile([128, 384], F32, tag="qkvps")
                        for ck in range(6):
                            nc.tensor.matmul(
                                ps[:],
                                xt[:, ck, :],
                                wq[:, ck, 0:384],
                                start=(ck == 0),
                                stop=(ck == 5),
                            )
                        b_copies(0, ps, nb, 0, nc.vector.tensor_copy)
                        # head 1's copies on the idle ACT engine
                        b_copies(1, ps, nb, 192, nc.scalar.copy)

            def phase_b2_mm(pB, psB, nb_lo, nb_hi):
                """Head 2 qkv matmuls + copies for a range of nb (filler)."""
                for nb in range(nb_lo, nb_hi):
                    xt = pB.tile([128, 6, 128], BF16, tag="xt2")
                    nc.sync.dma_start(
                        xt[:],
                        x_t.rearrange("(ck p) n -> p ck n", p=128)[
                            :, :, nb * 128 : (nb + 1) * 128
                        ],
                    )
                    ps = psB.tile([128, 192], F32, tag="qkvps2")
                    for ck in range(6):
                        nc.tensor.matmul(
                            ps[:],
                            xt[:, ck, :],
                            wq[:, ck, 384:576],
                            start=(ck == 0),
                            stop=(ck == 5),
                        )
                    b_copies(2, ps, nb, 0, nc.vector.tensor_copy)

            def t_k_pe(h, psT, cb_lo=0, cb_hi=None):
                """kT column blocks via PE transpose: rawk's (cb, jh, d)
                block [128,128] transposes straight into kT's stacked layout
                (16 transposes per head)."""
                for cb in range(cb_lo, min(cb_hi if cb_hi is not None else NHALF, NHALF)):
                    pk = psT.tile([128, 128], BF16, tag=f"tk{h}")
                    nc.tensor.transpose(
                        pk[:],
                        rawks[h][:, cb, :, :].rearrange("p j d -> p (j d)"),
                        identb[:],
                    )
                    nc.vector.tensor_copy(
                        kT[h][:, cb * 128 : (cb + 1) * 128], pk[:]
                    )

            def t_q(h, nb_lo, nb_hi):
                """qT blocks via 2D DMA transposes from the duplicated slab
                (issued on SP, paced a few per i-block)."""
                qdup = qdups[h]
                for nb in range(nb_lo, min(nb_hi, NB)):
                    nc.sync.dma_start_transpose(
                        qT[h][:, nb * 128 : (nb + 1) * 128],
                        qdup[:, nb, :, :],
                    )

            def proj_nb(psD, pD, nb_lo, nb_hi):
                """Output projection for a range of n-blocks (bf16)."""
                for nb in range(nb_lo, nb_hi):
                    blk = slice(nb * 128, (nb + 1) * 128)
                    stage = pD.tile([128, C], F32, tag="stage")
                    for oc, osz in ((0, 512), (512, 256)):
                        ps = psD.tile([128, 512], F32, tag="pd")
                        nc.tensor.matmul(
                            ps[:, 0:osz],
                            ao1[:, blk],
                            pw128[:, oc : oc + osz],
                            start=True,
                            stop=False,
                        )
                        nc.tensor.matmul(
                            ps[:, 0:osz],
                            ao2[0:64, blk],
                            pw64[0:64, oc : oc + osz],
                            start=False,
                            stop=True,
                        )
                        nc.vector.tensor_copy(stage[:, oc : oc + osz], ps[:, 0:osz])
                    nc.sync.dma_start(out_p[blk, :], stage[:])

            def phase_c(h, ptp, pCs, psS, psO, ib_hook=None):
                """Full attention for head h; ib_hook(ib) emits filler PE work.

                The score/PV PSUM pools are shared across heads so no pool
                boundary (which waits on ALL prior readers, including trailing
                exps) sits between consecutive heads."""
                if True:
                    ngrp = (NB + GRP - 1) // GRP
                    for ib in range(IB):
                        isl = slice(ib * 512, (ib + 1) * 512)
                        pso = psO.tile([65, 512], F32, tag="pso")
                        queue = []
                        n_pv = [0]

                        def emit_pv(pso=pso, queue=queue, n_pv=n_pv, h=h):
                            pt_half, jc = queue.pop(0)
                            nc.tensor.matmul(
                                pso[:],
                                vA[h][:, jc, :],
                                pt_half,
                                start=(n_pv[0] == 0),
                                stop=(n_pv[0] == NB - 1),
                            )
                            n_pv[0] += 1

                        for g in range(ngrp):
                            # pair chunks (g, g+NHALF): alternating PE tile
                            # rows lets the next LDWEIGHTS overlap the current
                            # matmul in the other quadrant
                            chunks = [g, g + NHALF]
                            w = len(chunks)
                            ps = psS.tile([128, GRP, 512], F32, tag="st")
                            for s in range(w):
                                jc = chunks[s]
                                jh, cb = jc // NHALF, jc % NHALF
                                psl = slice(64 * jh, 64 * jh + 64)
                                nc.tensor.matmul(
                                    ps[:, s, :],
                                    kT[h][psl, cb * 128 : (cb + 1) * 128],
                                    qT[h][psl, isl],
                                    start=True,
                                    stop=True,
                                    tile_position=(64 * jh, 0),
                                )
                            pt = ptp.tile([128, GRP, 512], BF16, tag="pt")
                            nc.scalar.activation(
                                pt[:, 0:w, :], ps[:, 0:w, :], ACTF.Exp
                            )
                            for s in range(w):
                                queue.append((pt[:, s, :], chunks[s]))
                            while len(queue) > SKEW:
                                emit_pv()
                        while queue:
                            emit_pv()

                        rden_f = pCs.tile([1, 512], F32, tag="rden_f")
                        nc.vector.tensor_copy(rden_f[:], pso[64:65, :])
                        rden = pCs.tile([1, 512], F32, tag="rden")
                        nc.vector.reciprocal_approx_fast(rden[:], rden_f[:])
                        rb = pCs.tile([64, 512], F32, tag="rb")
                        nc.gpsimd.partition_broadcast(rb[:], rden[:])
                        if h == 0:
                            nc.vector.tensor_mul(ao1[0:64, isl], pso[0:64, :], rb[:])
                        elif h == 2:
                            nc.vector.tensor_mul(ao2[0:64, isl], pso[0:64, :], rb[:])
                        else:
                            stg = pCs.tile([64, 512], BF16, tag="stg")
                            nc.vector.tensor_mul(stg[:], pso[0:64, :], rb[:])
                            nc.sync.dma_start(ao1[64:128, isl], stg[:])
                        if ib_hook is not None:
                            ib_hook(ib)

            # ================= emission =================
            phase_b01()
            mrs0 = ln_stats(0)
            ln_finish_k(0, *mrs0)
            ln_finish_q(0, *mrs0)
            alloc_raw(2)
            with tc.tile_pool(name="psTk0", bufs=2, space="PSUM") as psTk0:
                t_k_pe(0, psTk0)
            t_q(0, 0, 4)

            # shared attention pools: psS 4 banks + psO 2 = 6; extras <= 2
            with (
                tc.tile_pool(name="ptC", bufs=4) as ptp,
                tc.tile_pool(name="pCs", bufs=4) as pCs,
                tc.tile_pool(name="psS", bufs=2, space="PSUM") as psS,
                tc.tile_pool(name="psO", bufs=2, space="PSUM") as psO,
            ):
                with (
                    tc.tile_pool(name="pB2", bufs=4) as pB2,
                    tc.tile_pool(name="psB2", bufs=1, space="PSUM") as psB2,
                    tc.tile_pool(name="psTk1", bufs=1, space="PSUM") as psTk1,
                ):
                    def hook0(ib):
                        if ib == 0:
                            mrs1 = ln_stats(1)
                            ln_finish_k(1, *mrs1)
                            ln_finish_q(1, *mrs1)
                        elif ib in (2, 3):
                            t_k_pe(1, psTk1, (ib - 2) * 8, (ib - 1) * 8)
                        phase_b2_mm(pB2, psB2, ib * 4, ib * 4 + 4)
                        t_q(0, (ib + 1) * 4, (ib + 2) * 4)
                        t_q(1, ib * 4, (ib + 1) * 4)
                    phase_c(0, ptp, pCs, psS, psO, ib_hook=hook0)

                # head 2 LN overlaps C1 startup; its transposes fill C1
                mrs2 = ln_stats(2)
                ln_finish_k(2, *mrs2)
                ln_finish_q(2, *mrs2)
                with tc.tile_pool(name="psTk2", bufs=1, space="PSUM") as psTk2:
                    def hook1(ib):
                        if ib in (1, 2):
                            t_k_pe(2, psTk2, (ib - 1) * 8, ib * 8)
                        t_q(2, ib * 4, (ib + 1) * 4)
                    phase_c(1, ptp, pCs, psS, psO, ib_hook=hook1)

                with (
                    tc.tile_pool(name="pD", bufs=3) as pD,
                    tc.tile_pool(name="psD", bufs=2, space="PSUM") as psD,
                ):
                    def hook2(ib):
                        proj_nb(psD, pD, ib * 4, ib * 4 + 4)
                    phase_c(2, ptp, pCs, psS, psO, ib_hook=hook2)

    nc.compile()
    return nc


@lru_cache(maxsize=2)
def _built(N):
    nc = build_nc(N)
    return nc


def _prep_inputs(x, qkv_w, q_gamma, q_beta, k_gamma, k_beta, proj_w):
    x = np.asarray(x, np.float32)
    qkv_w = np.asarray(qkv_w, np.float32)
    proj_w = np.asarray(proj_w, np.float32)
    B = x.shape[0]
    import ml_dtypes
    xts = [np.ascontiguousarray(x[b].T).astype(ml_dtypes.bfloat16) for b in range(B)]
    gbs = []
    wqs = []
    pws = []
    for g in range(4):
        r = slice(192 * g, 192 * (g + 1))
        qg = qkv_w[0:768][r]       # [192, 768] q rows of this group's 3 heads
        kg = qkv_w[768:1536][r]
        vg = qkv_w[1536:2304][r]
        # per-head interleave: [q_h(64) | k_h(64) | v_h(64)] x 3 heads
        blocks = []
        for h in range(3):
            hs = slice(64 * h, 64 * (h + 1))
            blocks += [qg[hs], kg[hs], vg[hs]]
        wq_rows = np.concatenate(blocks, axis=0)   # [576, 768]
        wqs.append(np.ascontiguousarray(wq_rows.T).astype(ml_dtypes.bfloat16))
        pws.append(
            np.ascontiguousarray(proj_w[:, r].T).astype(ml_dtypes.bfloat16)
        )
        gbs.append(
            np.stack(
                [
                    np.asarray(q_gamma, np.float32) * SCALE,
                    np.asarray(q_beta, np.float32) * SCALE,
                    np.asarray(k_gamma, np.float32),
                    np.asarray(k_beta, np.float32),
                ]
            )
        )
    in_maps = []
    for core in range(8):
        b, g = core // 4, core % 4
        in_maps.append(
            {"x_t": xts[b], "wqkv_t": wqs[g], "projw_t": pws[g], "gb": gbs[g]}
        )
    return in_maps


def run_cores(in_maps, N, trace=False):
    from concourse.bass_utils import run_bass_kernel_spmd

    nc = _built(N)
    res = run_bass_kernel_spmd(nc, in_maps, list(range(8)), trace=trace)
    return res


def kernel(x, qkv_w, q_gamma, q_beta, k_gamma, k_beta, proj_w, proj_b):
    x = np.asarray(x, np.float32)
    N = x.shape[1]
    in_maps = _prep_inputs(x, qkv_w, q_gamma, q_beta, k_gamma, k_beta, proj_w)
    res = run_cores(in_maps, N)
    parts = [np.asarray(r["out_p"], np.float32) for r in res.results]
    out0 = parts[0] + parts[1] + parts[2] + parts[3]
    out1 = parts[4] + parts[5] + parts[6] + parts[7]
    out = np.stack([out0, out1]) + np.asarray(proj_b, np.float32)
    return out.astype(np.float32)


# revision 34
# speedup vs baseline: 1.2217x; 1.1656x over previous
"""Trainium2 Bass kernel for CustomAttention (qkv -> per-head LN on q,k -> SDPA -> proj).

Sharding: 8 cores = 2 batches x 4 head-groups (3 heads each).

v4 structure:
- Phase B for heads 0+1 together (384-row qkv matmuls); raw/v copies split
  across DVE and the otherwise-idle ACT engine.
- LN per head finishes k first (Pool mu/rstd passes split by type), so the
  32 k-transposes can run while q's LN completes; q-transposes are
  interleaved into the attention i-block loop (only the next i-block's
  columns are needed).
- Head 2's qkv matmuls interleave into head 0's attention; its LN runs
  during attention. Output projection (bf16) interleaves into head 2's
  attention loop.
- ACT runs exp on [128,1024] PSUM tiles; PSUM: scores 2x2 banks + PV 2x1 +
  transpose/filler pools.
"""

import os
import sys
from functools import lru_cache

import numpy as np

for _p in ("/opt/trn_rl_repo", os.path.expanduser("~/.axon_site/_ro/trn_rl_repo")):
    if os.path.isdir(_p) and _p not in sys.path:
        sys.path.insert(0, _p)

import concourse.bass as bass
import concourse.mybir as mybir
from concourse import bacc
import concourse.tile as tile
from concourse.masks import make_identity

F32 = mybir.dt.float32
F32R = mybir.dt.float32r
BF16 = mybir.dt.bfloat16
ALU = mybir.AluOpType
ACTF = mybir.ActivationFunctionType

H = 3          # heads per core
D = 64         # head dim
C = 768        # model dim
J = 3 * H * D  # qkv rows per core = 576
EPS = 1e-5
SCALE = D ** -0.5

SKEW = 6       # PV matmuls lag scores by this many 512-wide chunks
GRP = 2        # score j-chunks per PSUM/exp tile


def build_nc(N=4096):
    """One-core program; all 8 cores run it SPMD with different input data."""
    NB = N // 128          # n-blocks / j-chunks = 32
    IB = N // 512          # i-blocks = 8
    NHALF = NB // 2

    nc = bacc.Bacc("TRN2", target_bir_lowering=False, debug=False)
    x_t = nc.declare_dram_parameter("x_t", [C, N], BF16, isOutput=False)
    # host layout: [C, (h, q|k|v, 64)] = per-head column groups
    wqkv_t = nc.declare_dram_parameter("wqkv_t", [C, J], BF16, isOutput=False)
    projw_t = nc.declare_dram_parameter("projw_t", [H * D, C], BF16, isOutput=False)
    gb = nc.declare_dram_parameter("gb", [4, D], F32, isOutput=False)
    out_p = nc.declare_dram_parameter("out_p", [N, C], F32, isOutput=True)

    with tile.TileContext(nc) as tc:
        with (
            tc.tile_pool(name="persist", bufs=1) as persist,
            tc.tile_pool(name="weights", bufs=1) as weights,
            tc.tile_pool(name="raw", bufs=2) as rawp,
            tc.tile_pool(name="stats", bufs=2) as statsp,
        ):
            # ---- persistent SBUF tensors ----
            # qT duplicated across both partition halves: rows 0:64 == 64:128
            qT = [persist.tile([128, N], BF16, tag=f"qT{h}", name=f"qT{h}") for h in range(H)]
            # kT stacked: rows 0:64 = j in [0,N/2), rows 64:128 = j in [N/2,N)
            kT = [persist.tile([128, N // 2], BF16, tag=f"kT{h}", name=f"kT{h}") for h in range(H)]
            # V augmented with a ones column (index 64) per j-chunk
            vA = [persist.tile([128, NB, 65], BF16, tag=f"vA{h}", name=f"vA{h}") for h in range(H)]
            # attention output, channel-major: ao1 rows = h0,h1; ao2 rows = h2
            ao1 = persist.tile([128, N], BF16, tag="ao1")
            ao2 = persist.tile([64, N], BF16, tag="ao2")

            for h in range(H):
                nc.vector.memset(vA[h][:, :, 64:65], 1.0)

            wq = weights.tile([128, 6, J], BF16, tag="wqkv")
            nc.sync.dma_start(
                wq[:], wqkv_t.rearrange("(ck p) j -> p ck j", p=128)
            )
            pw128 = weights.tile([128, C], BF16, tag="pw128")
            nc.sync.dma_start(pw128[:], projw_t[0:128, :])
            pw64 = weights.tile([64, C], BF16, tag="pw64")
            nc.sync.dma_start(pw64[:], projw_t[128:192, :])
            # gamma/beta broadcast across partitions: rows [gq*s, bq*s, gk, bk]
            gbt = weights.tile([128, 4, D], F32, tag="gb")
            nc.sync.dma_start(gbt[:], gb[None, :, :].to_broadcast([128, 4, D]))
            gam2 = weights.tile([128, 2, D], BF16, tag="gam2")
            nc.vector.tensor_copy(gam2[:, 0, :], gbt[:, 0, :])
            nc.vector.tensor_copy(gam2[:, 1, :], gbt[:, 2, :])
            bet2 = weights.tile([128, 2, D], BF16, tag="bet2")
            nc.vector.tensor_copy(bet2[:, 0, :], gbt[:, 1, :])
            nc.vector.tensor_copy(bet2[:, 1, :], gbt[:, 3, :])
            epst = weights.tile([128, 1], F32, tag="epst")
            nc.vector.memset(epst[:], EPS)
            ident = weights.tile([128, 128], F32, tag="ident")
            make_identity(nc, ident[:])
            identb = weights.tile([128, 128], BF16, tag="identb")
            nc.vector.tensor_copy(identb[:], ident[:])

            rawqs = {}
            rawks = {}
            qdups = {}

            def ln_stats(h):
                """Batched LN stats for head h; mu/rstd -> bf16.

                q slab is [128, NB, D]; k slab is [128, NHALF, 2, D] in
                (column-block, j-half) order so each kT 128-column block is
                one 2D DMA transpose. Stats index 1 follows k's layout."""
                rawq, rawk = rawqs[h], rawks[h]
                s1 = statsp.tile([128, 2, NB], F32, tag="s1")
                s2 = statsp.tile([128, 2, NB], F32, tag="s2")
                sq = rawp.tile([128, 2, NB, D], BF16, tag="sq", name=f"sq{h}")
                kv = rawk[:].rearrange("p c j d -> p (c j) d")
                nc.vector.tensor_reduce(
                    s1[:, 0, :], rawq[:], mybir.AxisListType.X, ALU.add
                )
                nc.vector.tensor_reduce(s1[:, 1, :], kv, mybir.AxisListType.X, ALU.add)
                nc.vector.tensor_mul(sq[:, 0], rawq[:], rawq[:])
                nc.vector.tensor_mul(sq[:, 1], kv, kv)
                nc.vector.tensor_reduce(s2[:], sq[:], mybir.AxisListType.X, ALU.add)
                mu = statsp.tile([128, 2, NB], F32, tag="mu")
                nc.vector.tensor_scalar_mul(mu[:], s1[:], 1.0 / D)
                var = statsp.tile([128, 2, NB], F32, tag="var")
                nc.vector.tensor_scalar_mul(var[:], s2[:], 1.0 / D)
                musq = statsp.tile([128, 2, NB], F32, tag="musq")
                nc.vector.tensor_mul(musq[:], mu[:], mu[:])
                nc.vector.tensor_sub(var[:], var[:], musq[:])
                std = statsp.tile([128, 2, NB], F32, tag="std")
                nc.scalar.activation(std[:], var[:], ACTF.Sqrt, bias=epst[:])
                rstd = statsp.tile([128, 2, NB], F32, tag="rstd")
                nc.vector.reciprocal(rstd[:], std[:])
                # one Newton step: r <- r*(1.5 - 0.5*(var+eps)*r^2)
                nr = statsp.tile([128, 2, NB], F32, tag="nr")
                nc.vector.tensor_mul(nr[:], rstd[:], rstd[:])
                ve = statsp.tile([128, 2, NB], F32, tag="ve")
                nc.vector.tensor_scalar_add(ve[:], var[:], EPS)
                nc.vector.tensor_mul(nr[:], nr[:], ve[:])
                nc.vector.tensor_scalar(nr[:], nr[:], -0.5, 1.5, ALU.mult, ALU.add)
                nc.vector.tensor_mul(rstd[:], rstd[:], nr[:])
                mu_b = statsp.tile([128, 2, NB], BF16, tag="mu_b", name=f"mu_b{h}")
                nc.vector.tensor_copy(mu_b[:], mu[:])
                rstd_b = statsp.tile([128, 2, NB], BF16, tag="rstd_b", name=f"rstd_b{h}")
                nc.vector.tensor_copy(rstd_b[:], rstd[:])
                return mu_b, rstd_b

            def ln_finish_k(h, mu_b, rstd_b):
                kk = rawks[h][:].rearrange("p c j d -> p (c j) d")
                mu4 = mu_b[:, 1, :, None].broadcast_to([128, NB, D])
                rs4 = rstd_b[:, 1, :, None].broadcast_to([128, NB, D])
                nc.gpsimd.tensor_sub(kk, kk, mu4)
                nc.gpsimd.tensor_mul(kk, kk, rs4)
                gk = gam2[:, 1, None, :].broadcast_to([128, NB, D])
                bk = bet2[:, 1, None, :].broadcast_to([128, NB, D])
                nc.vector.tensor_mul(kk, kk, gk)
                nc.vector.tensor_add(kk, kk, bk)

            def ln_finish_q(h, mu_b, rstd_b):
                qq = rawqs[h][:]
                mu4 = mu_b[:, 0, :, None].broadcast_to([128, NB, D])
                rs4 = rstd_b[:, 0, :, None].broadcast_to([128, NB, D])
                nc.gpsimd.tensor_sub(qq, qq, mu4)
                nc.gpsimd.tensor_mul(qq, qq, rs4)
                qdup = rawp.tile([128, NB, 2, D], BF16, tag="qdup", name=f"qdup{h}")
                gq = gam2[:, 0, None, :].broadcast_to([128, NB, D])
                bq = bet2[:, 0, None, :].broadcast_to([128, NB, D])
                nc.vector.tensor_mul(qdup[:, :, 0, :], qq, gq)
                nc.vector.tensor_add(qdup[:, :, 0, :], qdup[:, :, 0, :], bq)
                nc.vector.tensor_copy(qdup[:, :, 1, :], qdup[:, :, 0, :])
                qdups[h] = qdup

            def alloc_raw(h):
                rawqs[h] = rawp.tile([128, NB, D], BF16, tag="rawq", name=f"rawq{h}")
                rawks[h] = rawp.tile(
                    [128, NHALF, 2, D], BF16, tag="rawk", name=f"rawk{h}"
                )

            def b_copies(h, ps, nb, qk_off, eng):
                """psum -> raw slabs; k goes to its (cb, jh) slot."""
                jh, cb = nb // NHALF, nb % NHALF
                eng(rawqs[h][:, nb, :], ps[:, qk_off : qk_off + 64])
                eng(rawks[h][:, cb, jh, :], ps[:, qk_off + 64 : qk_off + 128])
                eng(vA[h][:, nb, 0:64], ps[:, qk_off + 128 : qk_off + 192])

            def phase_b01():
                """qkv for heads 0 and 1 (one pass over x)."""
                alloc_raw(0)
                alloc_raw(1)
                with (
                    tc.tile_pool(name="pB01", bufs=4) as pB,
                    tc.tile_pool(name="psB01", bufs=4, space="PSUM") as psB,
                ):
                    for nb in range(NB):
                        xt = pB.tile([128, 6, 128], BF16, tag="xt")
                        nc.sync.dma_start(
                            xt[:],
                            x_t.rearrange("(ck p) n -> p ck n", p=128)[
                                :, :, nb * 128 : (nb + 1) * 128
                            ],
                        )
                        ps = psB.tile([128, 384], F32, tag="qkvps")
                        for ck in range(6):
                            nc.tensor.matmul(
                                ps[:],
                                xt[:, ck, :],
                                wq[:, ck, 0:384],
                                start=(ck == 0),
                                stop=(ck == 5),
                            )
                        b_copies(0, ps, nb, 0, nc.vector.tensor_copy)
                        # head 1's copies on the idle ACT engine
                        b_copies(1, ps, nb, 192, nc.scalar.copy)

            def phase_b2_mm(pB, psB, nb_lo, nb_hi):
                """Head 2 qkv matmuls + copies for a range of nb (filler)."""
                for nb in range(nb_lo, nb_hi):
                    xt = pB.tile([128, 6, 128], BF16, tag="xt2")
                    nc.sync.dma_start(
                        xt[:],
                        x_t.rearrange("(ck p) n -> p ck n", p=128)[
                            :, :, nb * 128 : (nb + 1) * 128
                        ],
                    )
                    ps = psB.tile([128, 192], F32, tag="qkvps2")
                    for ck in range(6):
                        nc.tensor.matmul(
                            ps[:],
                            xt[:, ck, :],
                            wq[:, ck, 384:576],
                            start=(ck == 0),
                            stop=(ck == 5),
                        )
                    b_copies(2, ps, nb, 0, nc.vector.tensor_copy)

            def t_k_pe(h, psT, cb_lo=0, cb_hi=None):
                """kT column blocks via PE transpose: rawk's (cb, jh, d)
                block [128,128] transposes straight into kT's stacked layout
                (16 transposes per head)."""
                for cb in range(cb_lo, min(cb_hi if cb_hi is not None else NHALF, NHALF)):
                    pk = psT.tile([128, 128], BF16, tag=f"tk{h}")
                    nc.tensor.transpose(
                        pk[:],
                        rawks[h][:, cb, :, :].rearrange("p j d -> p (j d)"),
                        identb[:],
                    )
                    nc.vector.tensor_copy(
                        kT[h][:, cb * 128 : (cb + 1) * 128], pk[:]
                    )

            def t_q_pe(h, psT, nb_lo, nb_hi, tag=None):
                """qT blocks via PE transpose of the duplicated slab."""
                qdup = qdups[h]
                for nb in range(nb_lo, min(nb_hi, NB)):
                    pq = psT.tile([128, 128], BF16, tag=tag or f"tq{h}")
                    nc.tensor.transpose(
                        pq[:],
                        qdup[:, nb, :, :].rearrange("p t d -> p (t d)"),
                        identb[:],
                    )
                    nc.vector.tensor_copy(
                        qT[h][:, nb * 128 : (nb + 1) * 128], pq[:]
                    )

            def proj_nb(psD, pD, nb_lo, nb_hi):
                """Output projection for a range of n-blocks (bf16)."""
                for nb in range(nb_lo, nb_hi):
                    blk = slice(nb * 128, (nb + 1) * 128)
                    stage = pD.tile([128, C], F32, tag="stage")
                    for oc, osz in ((0, 512), (512, 256)):
                        ps = psD.tile([128, 512], F32, tag="pd")
                        nc.tensor.matmul(
                            ps[:, 0:osz],
                            ao1[:, blk],
                            pw128[:, oc : oc + osz],
                            start=True,
                            stop=False,
                        )
                        nc.tensor.matmul(
                            ps[:, 0:osz],
                            ao2[0:64, blk],
                            pw64[0:64, oc : oc + osz],
                            start=False,
                            stop=True,
                        )
                        nc.vector.tensor_copy(stage[:, oc : oc + osz], ps[:, 0:osz])
                    nc.sync.dma_start(out_p[blk, :], stage[:])

            def phase_c(h, ptp, pCs, psS, psO, ib_hook=None):
                """Full attention for head h; ib_hook(ib) emits filler PE work.

                The score/PV PSUM pools are shared across heads so no pool
                boundary (which waits on ALL prior readers, including trailing
                exps) sits between consecutive heads."""
                if True:
                    ngrp = (NB + GRP - 1) // GRP
                    for ib in range(IB):
                        isl = slice(ib * 512, (ib + 1) * 512)
                        pso = psO.tile([65, 512], F32, tag="pso")
                        queue = []
                        n_pv = [0]

                        def emit_pv(pso=pso, queue=queue, n_pv=n_pv, h=h):
                            pt_half, jc = queue.pop(0)
                            nc.tensor.matmul(
                                pso[:],
                                vA[h][:, jc, :],
                                pt_half,
                                start=(n_pv[0] == 0),
                                stop=(n_pv[0] == NB - 1),
                            )
                            n_pv[0] += 1

                        for g in range(ngrp):
                            # pair chunks (g, g+NHALF): alternating PE tile
                            # rows lets the next LDWEIGHTS overlap the current
                            # matmul in the other quadrant
                            chunks = [g, g + NHALF]
                            w = len(chunks)
                            ps = psS.tile([128, GRP, 512], F32, tag="st")
                            for s in range(w):
                                jc = chunks[s]
                                jh, cb = jc // NHALF, jc % NHALF
                                psl = slice(64 * jh, 64 * jh + 64)
                                nc.tensor.matmul(
                                    ps[:, s, :],
                                    kT[h][psl, cb * 128 : (cb + 1) * 128],
                                    qT[h][psl, isl],
                                    start=True,
                                    stop=True,
                                    tile_position=(64 * jh, 0),
                                )
                            pt = ptp.tile([128, GRP, 512], BF16, tag="pt")
                            nc.scalar.activation(
                                pt[:, 0:w, :], ps[:, 0:w, :], ACTF.Exp
                            )
                            for s in range(w):
                                queue.append((pt[:, s, :], chunks[s]))
                            while len(queue) > SKEW:
                                emit_pv()
                        while queue:
                            emit_pv()

                        rden_f = pCs.tile([1, 512], F32, tag="rden_f")
                        nc.vector.tensor_copy(rden_f[:], pso[64:65, :])
                        rden = pCs.tile([1, 512], F32, tag="rden")
                        nc.vector.reciprocal_approx_fast(rden[:], rden_f[:])
                        rb = pCs.tile([64, 512], F32, tag="rb")
                        nc.gpsimd.partition_broadcast(rb[:], rden[:])
                        if h == 0:
                            nc.vector.tensor_mul(ao1[0:64, isl], pso[0:64, :], rb[:])
                        elif h == 2:
                            nc.vector.tensor_mul(ao2[0:64, isl], pso[0:64, :], rb[:])
                        else:
                            stg = pCs.tile([64, 512], BF16, tag="stg")
                            nc.vector.tensor_mul(stg[:], pso[0:64, :], rb[:])
                            nc.sync.dma_start(ao1[64:128, isl], stg[:])
                        if ib_hook is not None:
                            ib_hook(ib)

            # ================= emission =================
            phase_b01()
            mrs0 = ln_stats(0)
            ln_finish_k(0, *mrs0)
            ln_finish_q(0, *mrs0)
            alloc_raw(2)
            with tc.tile_pool(name="psTk0", bufs=2, space="PSUM") as psTk0:
                t_k_pe(0, psTk0)
                t_q_pe(0, psTk0, 0, 4, tag="tk0")

            # shared attention pools: psS 4 banks + psO 2 = 6; extras <= 2
            with (
                tc.tile_pool(name="ptC", bufs=4) as ptp,
                tc.tile_pool(name="pCs", bufs=4) as pCs,
                tc.tile_pool(name="psS", bufs=2, space="PSUM") as psS,
                tc.tile_pool(name="psO", bufs=2, space="PSUM") as psO,
            ):
                with (
                    tc.tile_pool(name="pB2", bufs=4) as pB2,
                    tc.tile_pool(name="psB2", bufs=1, space="PSUM") as psB2,
                    tc.tile_pool(name="psF1", bufs=1, space="PSUM") as psF1,
                ):
                    # head 1 transposes (16 k + first 4 q) as paced PE filler
                    t1_work = [
                        (lambda cb=cb: t_k_pe(1, psF1, cb, cb + 1))
                        for cb in range(NHALF)
                    ] + [
                        (lambda nb=nb: t_q_pe(1, psF1, nb, nb + 1, tag="tk1"))
                        for nb in range(4)
                    ]

                    def hook0(ib):
                        if ib == 0:
                            mrs1 = ln_stats(1)
                            ln_finish_k(1, *mrs1)
                            ln_finish_q(1, *mrs1)
                        phase_b2_mm(pB2, psB2, ib * 4, ib * 4 + 4)
                        t_q_pe(0, psF1, (ib + 1) * 4, (ib + 2) * 4, tag="tk1")
                        if ib >= 3:
                            for _ in range(4):
                                if t1_work:
                                    t1_work.pop(0)()
                    phase_c(0, ptp, pCs, psS, psO, ib_hook=hook0)
                    assert not t1_work

                # head 2 LN overlaps C1 startup; its transposes fill C1
                mrs2 = ln_stats(2)
                ln_finish_k(2, *mrs2)
                ln_finish_q(2, *mrs2)
                with tc.tile_pool(name="psF2", bufs=1, space="PSUM") as psF2:
                    t2_work = [
                        (lambda cb=cb: t_k_pe(2, psF2, cb, cb + 1))
                        for cb in range(NHALF)
                    ] + [
                        (lambda nb=nb: t_q_pe(2, psF2, nb, nb + 1, tag="tk2"))
                        for nb in range(NB)
                    ]

                    def hook1(ib):
                        t_q_pe(1, psF2, (ib + 1) * 4, (ib + 2) * 4, tag="tk2")
                        if ib >= 1:
                            for _ in range(7):
                                if t2_work:
                                    t2_work.pop(0)()
                    phase_c(1, ptp, pCs, psS, psO, ib_hook=hook1)
                    while t2_work:
                        t2_work.pop(0)()

                with (
                    tc.tile_pool(name="pD", bufs=3) as pD,
                    tc.tile_pool(name="psD", bufs=2, space="PSUM") as psD,
                ):
                    def hook2(ib):
                        proj_nb(psD, pD, ib * 4, ib * 4 + 4)
                    phase_c(2, ptp, pCs, psS, psO, ib_hook=hook2)

    nc.compile()
    return nc


@lru_cache(maxsize=2)
def _built(N):
    nc = build_nc(N)
    return nc


def _prep_inputs(x, qkv_w, q_gamma, q_beta, k_gamma, k_beta, proj_w):
    x = np.asarray(x, np.float32)
    qkv_w = np.asarray(qkv_w, np.float32)
    proj_w = np.asarray(proj_w, np.float32)
    B = x.shape[0]
    import ml_dtypes
    xts = [np.ascontiguousarray(x[b].T).astype(ml_dtypes.bfloat16) for b in range(B)]
    gbs = []
    wqs = []
    pws = []
    for g in range(4):
        r = slice(192 * g, 192 * (g + 1))
        qg = qkv_w[0:768][r]       # [192, 768] q rows of this group's 3 heads
        kg = qkv_w[768:1536][r]
        vg = qkv_w[1536:2304][r]
        # per-head interleave: [q_h(64) | k_h(64) | v_h(64)] x 3 heads
        blocks = []
        for h in range(3):
            hs = slice(64 * h, 64 * (h + 1))
            blocks += [qg[hs], kg[hs], vg[hs]]
        wq_rows = np.concatenate(blocks, axis=0)   # [576, 768]
        wqs.append(np.ascontiguousarray(wq_rows.T).astype(ml_dtypes.bfloat16))
        pws.append(
            np.ascontiguousarray(proj_w[:, r].T).astype(ml_dtypes.bfloat16)
        )
        gbs.append(
            np.stack(
                [
                    np.asarray(q_gamma, np.float32) * SCALE,
                    np.asarray(q_beta, np.float32) * SCALE,
                    np.asarray(k_gamma, np.float32),
                    np.asarray(k_beta, np.float32),
                ]
            )
        )
    in_maps = []
    for core in range(8):
        b, g = core // 4, core % 4
        in_maps.append(
            {"x_t": xts[b], "wqkv_t": wqs[g], "projw_t": pws[g], "gb": gbs[g]}
        )
    return in_maps


def run_cores(in_maps, N, trace=False):
    from concourse.bass_utils import run_bass_kernel_spmd

    nc = _built(N)
    res = run_bass_kernel_spmd(nc, in_maps, list(range(8)), trace=trace)
    return res


def kernel(x, qkv_w, q_gamma, q_beta, k_gamma, k_beta, proj_w, proj_b):
    x = np.asarray(x, np.float32)
    N = x.shape[1]
    in_maps = _prep_inputs(x, qkv_w, q_gamma, q_beta, k_gamma, k_beta, proj_w)
    res = run_cores(in_maps, N)
    parts = [np.asarray(r["out_p"], np.float32) for r in res.results]
    out0 = parts[0] + parts[1] + parts[2] + parts[3]
    out1 = parts[4] + parts[5] + parts[6] + parts[7]
    out = np.stack([out0, out1]) + np.asarray(proj_b, np.float32)
    return out.astype(np.float32)


# revision 35
# speedup vs baseline: 1.2319x; 1.0084x over previous
"""Trainium2 Bass kernel for CustomAttention (qkv -> per-head LN on q,k -> SDPA -> proj).

Sharding: 8 cores = 2 batches x 4 head-groups (3 heads each).

v4 structure:
- Phase B for heads 0+1 together (384-row qkv matmuls); raw/v copies split
  across DVE and the otherwise-idle ACT engine.
- LN per head finishes k first (Pool mu/rstd passes split by type), so the
  32 k-transposes can run while q's LN completes; q-transposes are
  interleaved into the attention i-block loop (only the next i-block's
  columns are needed).
- Head 2's qkv matmuls interleave into head 0's attention; its LN runs
  during attention. Output projection (bf16) interleaves into head 2's
  attention loop.
- ACT runs exp on [128,1024] PSUM tiles; PSUM: scores 2x2 banks + PV 2x1 +
  transpose/filler pools.
"""

import os
import sys
from functools import lru_cache

import numpy as np

for _p in ("/opt/trn_rl_repo", os.path.expanduser("~/.axon_site/_ro/trn_rl_repo")):
    if os.path.isdir(_p) and _p not in sys.path:
        sys.path.insert(0, _p)

import concourse.bass as bass
import concourse.mybir as mybir
from concourse import bacc
import concourse.tile as tile
from concourse.masks import make_identity

F32 = mybir.dt.float32
F32R = mybir.dt.float32r
BF16 = mybir.dt.bfloat16
ALU = mybir.AluOpType
ACTF = mybir.ActivationFunctionType

H = 3          # heads per core
D = 64         # head dim
C = 768        # model dim
J = 3 * H * D  # qkv rows per core = 576
EPS = 1e-5
SCALE = D ** -0.5

SKEW = 6       # PV matmuls lag scores by this many 512-wide chunks
GRP = 2        # score j-chunks per PSUM/exp tile


def build_nc(N=4096):
    """One-core program; all 8 cores run it SPMD with different input data."""
    NB = N // 128          # n-blocks / j-chunks = 32
    IB = N // 512          # i-blocks = 8
    NHALF = NB // 2

    nc = bacc.Bacc("TRN2", target_bir_lowering=False, debug=False)
    x_t = nc.declare_dram_parameter("x_t", [C, N], BF16, isOutput=False)
    # host layout: [C, (h, q|k|v, 64)] = per-head column groups
    wqkv_t = nc.declare_dram_parameter("wqkv_t", [C, J], BF16, isOutput=False)
    projw_t = nc.declare_dram_parameter("projw_t", [H * D, C], BF16, isOutput=False)
    gb = nc.declare_dram_parameter("gb", [4, D], F32, isOutput=False)
    out_p = nc.declare_dram_parameter("out_p", [N, C], F32, isOutput=True)

    with tile.TileContext(nc) as tc:
        with (
            tc.tile_pool(name="persist", bufs=1) as persist,
            tc.tile_pool(name="weights", bufs=1) as weights,
            tc.tile_pool(name="raw", bufs=2) as rawp,
            tc.tile_pool(name="stats", bufs=2) as statsp,
        ):
            # ---- persistent SBUF tensors ----
            # qT duplicated across both partition halves: rows 0:64 == 64:128
            qT = [persist.tile([128, N], BF16, tag=f"qT{h}", name=f"qT{h}") for h in range(H)]
            # kT stacked: rows 0:64 = j in [0,N/2), rows 64:128 = j in [N/2,N)
            kT = [persist.tile([128, N // 2], BF16, tag=f"kT{h}", name=f"kT{h}") for h in range(H)]
            # V augmented with a ones column (index 64) per j-chunk
            vA = [persist.tile([128, NB, 65], BF16, tag=f"vA{h}", name=f"vA{h}") for h in range(H)]
            # attention output, channel-major: ao1 rows = h0,h1; ao2 rows = h2
            ao1 = persist.tile([128, N], BF16, tag="ao1")
            ao2 = persist.tile([64, N], BF16, tag="ao2")

            for h in range(H):
                nc.vector.memset(vA[h][:, :, 64:65], 1.0)

            wq = weights.tile([128, 6, J], BF16, tag="wqkv")
            nc.sync.dma_start(
                wq[:], wqkv_t.rearrange("(ck p) j -> p ck j", p=128)
            )
            pw128 = weights.tile([128, C], BF16, tag="pw128")
            nc.sync.dma_start(pw128[:], projw_t[0:128, :])
            pw64 = weights.tile([64, C], BF16, tag="pw64")
            nc.sync.dma_start(pw64[:], projw_t[128:192, :])
            # gamma/beta broadcast across partitions: rows [gq*s, bq*s, gk, bk]
            gbt = weights.tile([128, 4, D], F32, tag="gb")
            nc.sync.dma_start(gbt[:], gb[None, :, :].to_broadcast([128, 4, D]))
            gam2 = weights.tile([128, 2, D], BF16, tag="gam2")
            nc.vector.tensor_copy(gam2[:, 0, :], gbt[:, 0, :])
            nc.vector.tensor_copy(gam2[:, 1, :], gbt[:, 2, :])
            bet2 = weights.tile([128, 2, D], BF16, tag="bet2")
            nc.vector.tensor_copy(bet2[:, 0, :], gbt[:, 1, :])
            nc.vector.tensor_copy(bet2[:, 1, :], gbt[:, 3, :])
            epst = weights.tile([128, 1], F32, tag="epst")
            nc.vector.memset(epst[:], EPS)
            ident = weights.tile([128, 128], F32, tag="ident")
            make_identity(nc, ident[:])
            identb = weights.tile([128, 128], BF16, tag="identb")
            nc.vector.tensor_copy(identb[:], ident[:])

            rawqs = {}
            rawks = {}
            qdups = {}

            def ln_stats(h):
                """Batched LN stats for head h; mu/rstd -> bf16.

                q slab is [128, NB, D]; k slab is [128, NHALF, 2, D] in
                (column-block, j-half) order so each kT 128-column block is
                one 2D DMA transpose. Stats index 1 follows k's layout."""
                rawq, rawk = rawqs[h], rawks[h]
                s1 = statsp.tile([128, 2, NB], F32, tag="s1")
                s2 = statsp.tile([128, 2, NB], F32, tag="s2")
                sq = rawp.tile([128, 2, NB, D], BF16, tag="sq", name=f"sq{h}")
                kv = rawk[:].rearrange("p c j d -> p (c j) d")
                nc.vector.tensor_reduce(
                    s1[:, 0, :], rawq[:], mybir.AxisListType.X, ALU.add
                )
                nc.vector.tensor_reduce(s1[:, 1, :], kv, mybir.AxisListType.X, ALU.add)
                nc.vector.tensor_mul(sq[:, 0], rawq[:], rawq[:])
                nc.vector.tensor_mul(sq[:, 1], kv, kv)
                nc.vector.tensor_reduce(s2[:], sq[:], mybir.AxisListType.X, ALU.add)
                mu = statsp.tile([128, 2, NB], F32, tag="mu")
                nc.vector.tensor_scalar_mul(mu[:], s1[:], 1.0 / D)
                var = statsp.tile([128, 2, NB], F32, tag="var")
                nc.vector.tensor_scalar_mul(var[:], s2[:], 1.0 / D)
                musq = statsp.tile([128, 2, NB], F32, tag="musq")
                nc.vector.tensor_mul(musq[:], mu[:], mu[:])
                nc.vector.tensor_sub(var[:], var[:], musq[:])
                std = statsp.tile([128, 2, NB], F32, tag="std")
                nc.scalar.activation(std[:], var[:], ACTF.Sqrt, bias=epst[:])
                rstd = statsp.tile([128, 2, NB], F32, tag="rstd")
                nc.vector.reciprocal(rstd[:], std[:])
                # one Newton step: r <- r*(1.5 - 0.5*(var+eps)*r^2)
                nr = statsp.tile([128, 2, NB], F32, tag="nr")
                nc.vector.tensor_mul(nr[:], rstd[:], rstd[:])
                ve = statsp.tile([128, 2, NB], F32, tag="ve")
                nc.vector.tensor_scalar_add(ve[:], var[:], EPS)
                nc.vector.tensor_mul(nr[:], nr[:], ve[:])
                nc.vector.tensor_scalar(nr[:], nr[:], -0.5, 1.5, ALU.mult, ALU.add)
                nc.vector.tensor_mul(rstd[:], rstd[:], nr[:])
                mu_b = statsp.tile([128, 2, NB], BF16, tag="mu_b", name=f"mu_b{h}")
                nc.vector.tensor_copy(mu_b[:], mu[:])
                rstd_b = statsp.tile([128, 2, NB], BF16, tag="rstd_b", name=f"rstd_b{h}")
                nc.vector.tensor_copy(rstd_b[:], rstd[:])
                return mu_b, rstd_b

            def ln_stats_t(h, t):
                """LN stats for ONE type (t=0 q, t=1 k) of head h."""
                if t == 0:
                    src_ap = rawqs[h][:]
                else:
                    src_ap = rawks[h][:].rearrange("p c j d -> p (c j) d")
                s1 = statsp.tile([128, NB], F32, tag=f"s1t{t}")
                s2 = statsp.tile([128, NB], F32, tag=f"s2t{t}")
                sq = rawp.tile([128, NB, D], BF16, tag=f"sqt{t}")
                nc.vector.tensor_reduce(s1[:], src_ap, mybir.AxisListType.X, ALU.add)
                nc.vector.tensor_mul(sq[:], src_ap, src_ap)
                nc.vector.tensor_reduce(s2[:], sq[:], mybir.AxisListType.X, ALU.add)
                mu = statsp.tile([128, NB], F32, tag=f"mut{t}")
                nc.vector.tensor_scalar_mul(mu[:], s1[:], 1.0 / D)
                var = statsp.tile([128, NB], F32, tag=f"vart{t}")
                nc.vector.tensor_scalar_mul(var[:], s2[:], 1.0 / D)
                musq = statsp.tile([128, NB], F32, tag=f"musqt{t}")
                nc.vector.tensor_mul(musq[:], mu[:], mu[:])
                nc.vector.tensor_sub(var[:], var[:], musq[:])
                std = statsp.tile([128, NB], F32, tag=f"stdt{t}")
                nc.scalar.activation(std[:], var[:], ACTF.Sqrt, bias=epst[:])
                rstd = statsp.tile([128, NB], F32, tag=f"rstdt{t}")
                nc.vector.reciprocal(rstd[:], std[:])
                nr = statsp.tile([128, NB], F32, tag=f"nrt{t}")
                nc.vector.tensor_mul(nr[:], rstd[:], rstd[:])
                ve = statsp.tile([128, NB], F32, tag=f"vet{t}")
                nc.vector.tensor_scalar_add(ve[:], var[:], EPS)
                nc.vector.tensor_mul(nr[:], nr[:], ve[:])
                nc.vector.tensor_scalar(nr[:], nr[:], -0.5, 1.5, ALU.mult, ALU.add)
                nc.vector.tensor_mul(rstd[:], rstd[:], nr[:])
                mu_b = statsp.tile([128, NB], BF16, tag=f"mu_bt{t}")
                nc.vector.tensor_copy(mu_b[:], mu[:])
                rstd_b = statsp.tile([128, NB], BF16, tag=f"rstd_bt{t}")
                nc.vector.tensor_copy(rstd_b[:], rstd[:])
                return mu_b, rstd_b

            def ln_finish_k(h, mu2, rs2):
                kk = rawks[h][:].rearrange("p c j d -> p (c j) d")
                mu4 = mu2[:, :, None].broadcast_to([128, NB, D])
                rs4 = rs2[:, :, None].broadcast_to([128, NB, D])
                nc.gpsimd.tensor_sub(kk, kk, mu4)
                nc.gpsimd.tensor_mul(kk, kk, rs4)
                gk = gam2[:, 1, None, :].broadcast_to([128, NB, D])
                bk = bet2[:, 1, None, :].broadcast_to([128, NB, D])
                nc.vector.tensor_mul(kk, kk, gk)
                nc.vector.tensor_add(kk, kk, bk)

            def ln_finish_q(h, mu2, rs2):
                qq = rawqs[h][:]
                mu4 = mu2[:, :, None].broadcast_to([128, NB, D])
                rs4 = rs2[:, :, None].broadcast_to([128, NB, D])
                nc.gpsimd.tensor_sub(qq, qq, mu4)
                nc.gpsimd.tensor_mul(qq, qq, rs4)
                qdup = rawp.tile([128, NB, 2, D], BF16, tag="qdup", name=f"qdup{h}")
                gq = gam2[:, 0, None, :].broadcast_to([128, NB, D])
                bq = bet2[:, 0, None, :].broadcast_to([128, NB, D])
                nc.vector.tensor_mul(qdup[:, :, 0, :], qq, gq)
                nc.vector.tensor_add(qdup[:, :, 0, :], qdup[:, :, 0, :], bq)
                nc.vector.tensor_copy(qdup[:, :, 1, :], qdup[:, :, 0, :])
                qdups[h] = qdup

            def alloc_raw(h):
                rawqs[h] = rawp.tile([128, NB, D], BF16, tag="rawq", name=f"rawq{h}")
                rawks[h] = rawp.tile(
                    [128, NHALF, 2, D], BF16, tag="rawk", name=f"rawk{h}"
                )

            def b_copies(h, ps, nb, qk_off, eng):
                """psum -> raw slabs; k goes to its (cb, jh) slot."""
                jh, cb = nb // NHALF, nb % NHALF
                eng(rawqs[h][:, nb, :], ps[:, qk_off : qk_off + 64])
                eng(rawks[h][:, cb, jh, :], ps[:, qk_off + 64 : qk_off + 128])
                eng(vA[h][:, nb, 0:64], ps[:, qk_off + 128 : qk_off + 192])

            def phase_b01():
                """qkv for heads 0 and 1 (one pass over x)."""
                alloc_raw(0)
                alloc_raw(1)
                with (
                    tc.tile_pool(name="pB01", bufs=4) as pB,
                    tc.tile_pool(name="psB01", bufs=4, space="PSUM") as psB,
                ):
                    for nb in range(NB):
                        xt = pB.tile([128, 6, 128], BF16, tag="xt")
                        nc.sync.dma_start(
                            xt[:],
                            x_t.rearrange("(ck p) n -> p ck n", p=128)[
                                :, :, nb * 128 : (nb + 1) * 128
                            ],
                        )
                        ps = psB.tile([128, 384], F32, tag="qkvps")
                        for ck in range(6):
                            nc.tensor.matmul(
                                ps[:],
                                xt[:, ck, :],
                                wq[:, ck, 0:384],
                                start=(ck == 0),
                                stop=(ck == 5),
                            )
                        b_copies(0, ps, nb, 0, nc.vector.tensor_copy)
                        # head 1's copies on the idle ACT engine
                        b_copies(1, ps, nb, 192, nc.scalar.copy)

            def phase_b2_mm(pB, psB, nb_lo, nb_hi):
                """Head 2 qkv matmuls + copies for a range of nb (filler)."""
                for nb in range(nb_lo, nb_hi):
                    xt = pB.tile([128, 6, 128], BF16, tag="xt2")
                    nc.sync.dma_start(
                        xt[:],
                        x_t.rearrange("(ck p) n -> p ck n", p=128)[
                            :, :, nb * 128 : (nb + 1) * 128
                        ],
                    )
                    ps = psB.tile([128, 192], F32, tag="qkvps2")
                    for ck in range(6):
                        nc.tensor.matmul(
                            ps[:],
                            xt[:, ck, :],
                            wq[:, ck, 384:576],
                            start=(ck == 0),
                            stop=(ck == 5),
                        )
                    b_copies(2, ps, nb, 0, nc.vector.tensor_copy)

            def t_k_pe(h, psT, cb_lo=0, cb_hi=None):
                """kT column blocks via PE transpose: rawk's (cb, jh, d)
                block [128,128] transposes straight into kT's stacked layout
                (16 transposes per head)."""
                for cb in range(cb_lo, min(cb_hi if cb_hi is not None else NHALF, NHALF)):
                    pk = psT.tile([128, 128], BF16, tag=f"tk{h}")
                    nc.tensor.transpose(
                        pk[:],
                        rawks[h][:, cb, :, :].rearrange("p j d -> p (j d)"),
                        identb[:],
                    )
                    nc.vector.tensor_copy(
                        kT[h][:, cb * 128 : (cb + 1) * 128], pk[:]
                    )

            def t_q_pe(h, psT, nb_lo, nb_hi, tag=None):
                """qT blocks via PE transpose of the duplicated slab."""
                qdup = qdups[h]
                for nb in range(nb_lo, min(nb_hi, NB)):
                    pq = psT.tile([128, 128], BF16, tag=tag or f"tq{h}")
                    nc.tensor.transpose(
                        pq[:],
                        qdup[:, nb, :, :].rearrange("p t d -> p (t d)"),
                        identb[:],
                    )
                    nc.vector.tensor_copy(
                        qT[h][:, nb * 128 : (nb + 1) * 128], pq[:]
                    )

            def proj_nb(psD, pD, nb_lo, nb_hi):
                """Output projection for a range of n-blocks (bf16)."""
                for nb in range(nb_lo, nb_hi):
                    blk = slice(nb * 128, (nb + 1) * 128)
                    stage = pD.tile([128, C], F32, tag="stage")
                    for oc, osz in ((0, 512), (512, 256)):
                        ps = psD.tile([128, 512], F32, tag="pd")
                        nc.tensor.matmul(
                            ps[:, 0:osz],
                            ao1[:, blk],
                            pw128[:, oc : oc + osz],
                            start=True,
                            stop=False,
                        )
                        nc.tensor.matmul(
                            ps[:, 0:osz],
                            ao2[0:64, blk],
                            pw64[0:64, oc : oc + osz],
                            start=False,
                            stop=True,
                        )
                        nc.vector.tensor_copy(stage[:, oc : oc + osz], ps[:, 0:osz])
                    nc.sync.dma_start(out_p[blk, :], stage[:])

            def phase_c(h, ptp, pCs, psS, psO, ib_hook=None):
                """Full attention for head h; ib_hook(ib) emits filler PE work.

                The score/PV PSUM pools are shared across heads so no pool
                boundary (which waits on ALL prior readers, including trailing
                exps) sits between consecutive heads."""
                if True:
                    ngrp = (NB + GRP - 1) // GRP
                    for ib in range(IB):
                        isl = slice(ib * 512, (ib + 1) * 512)
                        pso = psO.tile([65, 512], F32, tag="pso")
                        queue = []
                        n_pv = [0]

                        def emit_pv(pso=pso, queue=queue, n_pv=n_pv, h=h):
                            pt_half, jc = queue.pop(0)
                            nc.tensor.matmul(
                                pso[:],
                                vA[h][:, jc, :],
                                pt_half,
                                start=(n_pv[0] == 0),
                                stop=(n_pv[0] == NB - 1),
                            )
                            n_pv[0] += 1

                        for g in range(ngrp):
                            # pair chunks (g, g+NHALF): alternating PE tile
                            # rows lets the next LDWEIGHTS overlap the current
                            # matmul in the other quadrant
                            chunks = [g, g + NHALF]
                            w = len(chunks)
                            ps = psS.tile([128, GRP, 512], F32, tag="st")
                            for s in range(w):
                                jc = chunks[s]
                                jh, cb = jc // NHALF, jc % NHALF
                                psl = slice(64 * jh, 64 * jh + 64)
                                nc.tensor.matmul(
                                    ps[:, s, :],
                                    kT[h][psl, cb * 128 : (cb + 1) * 128],
                                    qT[h][psl, isl],
                                    start=True,
                                    stop=True,
                                    tile_position=(64 * jh, 0),
                                )
                            pt = ptp.tile([128, GRP, 512], BF16, tag="pt")
                            nc.scalar.activation(
                                pt[:, 0:w, :], ps[:, 0:w, :], ACTF.Exp
                            )
                            for s in range(w):
                                queue.append((pt[:, s, :], chunks[s]))
                            while len(queue) > SKEW:
                                emit_pv()
                        while queue:
                            emit_pv()

                        rden_f = pCs.tile([1, 512], F32, tag="rden_f")
                        nc.vector.tensor_copy(rden_f[:], pso[64:65, :])
                        rden = pCs.tile([1, 512], F32, tag="rden")
                        nc.vector.reciprocal_approx_fast(rden[:], rden_f[:])
                        rb = pCs.tile([64, 512], F32, tag="rb")
                        nc.gpsimd.partition_broadcast(rb[:], rden[:])
                        if h == 0:
                            nc.vector.tensor_mul(ao1[0:64, isl], pso[0:64, :], rb[:])
                        elif h == 2:
                            nc.vector.tensor_mul(ao2[0:64, isl], pso[0:64, :], rb[:])
                        else:
                            stg = pCs.tile([64, 512], BF16, tag="stg")
                            nc.vector.tensor_mul(stg[:], pso[0:64, :], rb[:])
                            nc.sync.dma_start(ao1[64:128, isl], stg[:])
                        if ib_hook is not None:
                            ib_hook(ib)

            # ================= emission =================
            phase_b01()
            # head 0 LN: k chain first so its transposes (which gate C0)
            # start ASAP; q chain runs behind it on Pool/DVE
            muk, rsk = ln_stats_t(0, 1)
            ln_finish_k(0, muk, rsk)
            muq, rsq = ln_stats_t(0, 0)
            alloc_raw(2)
            with tc.tile_pool(name="psTk0", bufs=2, space="PSUM") as psTk0:
                t_k_pe(0, psTk0)
                ln_finish_q(0, muq, rsq)
                t_q_pe(0, psTk0, 0, 4, tag="tk0")

            # shared attention pools: psS 4 banks + psO 2 = 6; extras <= 2
            with (
                tc.tile_pool(name="ptC", bufs=4) as ptp,
                tc.tile_pool(name="pCs", bufs=4) as pCs,
                tc.tile_pool(name="psS", bufs=2, space="PSUM") as psS,
                tc.tile_pool(name="psO", bufs=2, space="PSUM") as psO,
            ):
                with (
                    tc.tile_pool(name="pB2", bufs=4) as pB2,
                    tc.tile_pool(name="psB2", bufs=1, space="PSUM") as psB2,
                    tc.tile_pool(name="psF1", bufs=1, space="PSUM") as psF1,
                ):
                    # head 1 transposes (16 k + first 4 q) as paced PE filler
                    t1_work = [
                        (lambda nb=nb: t_q_pe(1, psF1, nb, nb + 1, tag="tk1"))
                        for nb in range(4)
                    ] + [
                        (lambda cb=cb: t_k_pe(1, psF1, cb, cb + 1))
                        for cb in range(NHALF)
                    ]

                    def hook0(ib):
                        if ib == 0:
                            mu1, rs1 = ln_stats(1)
                            ln_finish_k(1, mu1[:, 1, :], rs1[:, 1, :])
                            ln_finish_q(1, mu1[:, 0, :], rs1[:, 0, :])
                        phase_b2_mm(pB2, psB2, ib * 4, ib * 4 + 4)
                        t_q_pe(0, psF1, (ib + 1) * 4, (ib + 2) * 4, tag="tk1")
                        if ib >= 3:
                            for _ in range(5):
                                if t1_work:
                                    t1_work.pop(0)()
                    phase_c(0, ptp, pCs, psS, psO, ib_hook=hook0)
                    assert not t1_work

                # head 2 LN overlaps C1 startup; its transposes fill C1
                mu2_, rs2_ = ln_stats(2)
                ln_finish_k(2, mu2_[:, 1, :], rs2_[:, 1, :])
                ln_finish_q(2, mu2_[:, 0, :], rs2_[:, 0, :])
                with tc.tile_pool(name="psF2", bufs=1, space="PSUM") as psF2:
                    t2_work = [
                        (lambda nb=nb: t_q_pe(2, psF2, nb, nb + 1, tag="tk2"))
                        for nb in range(4)
                    ] + [
                        (lambda cb=cb: t_k_pe(2, psF2, cb, cb + 1))
                        for cb in range(NHALF)
                    ] + [
                        (lambda nb=nb: t_q_pe(2, psF2, nb, nb + 1, tag="tk2"))
                        for nb in range(4, NB)
                    ]

                    def hook1(ib):
                        t_q_pe(1, psF2, (ib + 1) * 4, (ib + 2) * 4, tag="tk2")
                        if ib >= 1:
                            for _ in range(7):
                                if t2_work:
                                    t2_work.pop(0)()
                    phase_c(1, ptp, pCs, psS, psO, ib_hook=hook1)
                    while t2_work:
                        t2_work.pop(0)()

                with (
                    tc.tile_pool(name="pD", bufs=3) as pD,
                    tc.tile_pool(name="psD", bufs=2, space="PSUM") as psD,
                ):
                    def hook2(ib):
                        proj_nb(psD, pD, ib * 4, ib * 4 + 4)
                    phase_c(2, ptp, pCs, psS, psO, ib_hook=hook2)

    nc.compile()
    return nc


@lru_cache(maxsize=2)
def _built(N):
    nc = build_nc(N)
    return nc


def _prep_inputs(x, qkv_w, q_gamma, q_beta, k_gamma, k_beta, proj_w):
    x = np.asarray(x, np.float32)
    qkv_w = np.asarray(qkv_w, np.float32)
    proj_w = np.asarray(proj_w, np.float32)
    B = x.shape[0]
    import ml_dtypes
    xts = [np.ascontiguousarray(x[b].T).astype(ml_dtypes.bfloat16) for b in range(B)]
    gbs = []
    wqs = []
    pws = []
    for g in range(4):
        r = slice(192 * g, 192 * (g + 1))
        qg = qkv_w[0:768][r]       # [192, 768] q rows of this group's 3 heads
        kg = qkv_w[768:1536][r]
        vg = qkv_w[1536:2304][r]
        # per-head interleave: [q_h(64) | k_h(64) | v_h(64)] x 3 heads
        blocks = []
        for h in range(3):
            hs = slice(64 * h, 64 * (h + 1))
            blocks += [qg[hs], kg[hs], vg[hs]]
        wq_rows = np.concatenate(blocks, axis=0)   # [576, 768]
        wqs.append(np.ascontiguousarray(wq_rows.T).astype(ml_dtypes.bfloat16))
        pws.append(
            np.ascontiguousarray(proj_w[:, r].T).astype(ml_dtypes.bfloat16)
        )
        gbs.append(
            np.stack(
                [
                    np.asarray(q_gamma, np.float32) * SCALE,
                    np.asarray(q_beta, np.float32) * SCALE,
                    np.asarray(k_gamma, np.float32),
                    np.asarray(k_beta, np.float32),
                ]
            )
        )
    in_maps = []
    for core in range(8):
        b, g = core // 4, core % 4
        in_maps.append(
            {"x_t": xts[b], "wqkv_t": wqs[g], "projw_t": pws[g], "gb": gbs[g]}
        )
    return in_maps


def run_cores(in_maps, N, trace=False):
    from concourse.bass_utils import run_bass_kernel_spmd

    nc = _built(N)
    res = run_bass_kernel_spmd(nc, in_maps, list(range(8)), trace=trace)
    return res


def kernel(x, qkv_w, q_gamma, q_beta, k_gamma, k_beta, proj_w, proj_b):
    x = np.asarray(x, np.float32)
    N = x.shape[1]
    in_maps = _prep_inputs(x, qkv_w, q_gamma, q_beta, k_gamma, k_beta, proj_w)
    res = run_cores(in_maps, N)
    parts = [np.asarray(r["out_p"], np.float32) for r in res.results]
    out0 = parts[0] + parts[1] + parts[2] + parts[3]
    out1 = parts[4] + parts[5] + parts[6] + parts[7]
    out = np.stack([out0, out1]) + np.asarray(proj_b, np.float32)
    return out.astype(np.float32)
